# revision 72
# baseline (speedup 1.0000x reference)
"""Trainium2 Bass kernel for the SG-visibility sampling network.

Math notes (exploited structure):
  - U,V are orthogonal to the unit lobe axis l, so dot(sample_dir, l) == cos(r_phi)
    exactly (up to fp eps).  Hence the SG weight w = exp(sharp*(cos_phi-1)) is a
    per-lobe constant and sum_s(vis*w)/(sum_s w + TINY) = scale_l * sum_s vis with
    scale_l = w/(S*w + TINY), precomputed on host.
  - pre-activation of the hidden layer decomposes as
        pre_h[n,l,s,h] = P_n[h] - C_l[h] - ct[n,l,s]*A_l[h] - st[n,l,s]*B_l[h]
    with P_n = p_n @ W1[:3] + b1,  A_l = sp_l*(U_l@Wd),  B_l = sp_l*(V_l@Wd),
    C_l = cp_l*(l_l@Wd),  Wd = root_rot @ W1[3:].
  - hemisphere mask: cos_term = ct*a_nl + st*b_nl + c_nl with
    a = normals@(sp*U)_l, b = normals@(sp*V)_l, c = normals@(cp*l)_l.
  - sigmoid(z) = (1 + tanh(z/2)) / 2; the 1/2 is folded into the final
    per-lobe scale so Sin/Tanh/Relu all live in one ACT function table
    (silu_and_others) -> no per-chunk table reloads.

Device schedule (per core, data-parallel over N; fp32r on the PE where the
precision is irrelevant, full fp32 on the hemisphere-mask sign path):
  - theta streams through 4 rotating resident tiles, prefetched two chunks
    ahead on the Pool(SWDGE) DMA queue; setup DMAs are ordered by chunk-0
    criticality and spread across the SP/ACT/Pool queues.
  - a/b/c mask dots are one [128, 3*NC] tile in [l, n] layout (6 tiny fp32
    matmuls at setup); per chunk 3 DMAs s-duplicate them (split so the mask
    chain starts as soon as 'a' lands).  Mask products/sums run on the Pool
    engine (TensorTensor add/mult only — TSP and PSUM access are illegal
    there), the is_gt compare on DVE.
  - theta block layout (lobe8, ct/st interleaved, s8): ONE merged DMA per
    sub-chunk duplicates the theta rows (row pairs via an inner 0-stride
    AP dim); ONE Sin with per-partition phase bias emits the fp32r cos|sin
    moving operand.  cst tiles ping-pong between two persistent sets whose
    W1p/point rows are staged once at setup (fp32r needs a rounding
    producer, so everything f32r is written by DVE/Pool copies or ACT).
  - per-lobe: K<=116 fp32r matmul (zero-padded stationary) accumulates the
    hidden-layer pre-activation, relu (~19/13 DVE/ACT split, interleaved),
    K=128 fp32r matmul against a block-diag W2 accumulating all 16 lobes
    into one [128, HF] PSUM tile, ONE tanh per half, (1+t)*msk on Pool, and
    a final fp32r matmul applying 0.5*scale_l and summing over s into the
    [l, n] PSUM accumulator.
  - post-compile surgery keeps a single act-table load (set 18 holds Sin,
    Relu and Tanh together; the auto-inserted per-function loads would
    otherwise reload 1.28us tables 38 times).
"""

import numpy as np

N, L, S, H = 8192, 128, 8, 16
NCORES = 8
NC = N // NCORES          # rays per core
LPC = 16                  # lobes per super-chunk
CHUNKS = L // LPC
TINY = 1e-6

_PROG = None


def _build_program():
    import concourse.bass as bass
    import concourse.bacc as bacc
    import concourse.mybir as mybir
    import concourse.tile as tile

    f32 = mybir.dt.float32
    f32r = mybir.dt.float32r
    AF = mybir.ActivationFunctionType
    ALU = mybir.AluOpType
    PI4 = float(np.pi / 4.0)

    nc = bacc.Bacc("TRN2", target_bir_lowering=False, debug=False,
                   num_devices=NCORES)

    rt = nc.declare_dram_parameter("rt", [L * S, NC], f32, isOutput=False)
    nrmT = nc.declare_dram_parameter("nrmT", [3, NC], f32, isOutput=False)
    pc = nc.declare_dram_parameter("pc", [4, NC], f32, isOutput=False)
    wcst = nc.declare_dram_parameter("wcst", [128, L * 128], f32, isOutput=False)
    wabc = nc.declare_dram_parameter("wabc", [3, 3 * L], f32, isOutput=False)
    wsig = nc.declare_dram_parameter("wsig", [128, 2048], f32, isOutput=False)
    wsum = nc.declare_dram_parameter("wsum", [128, CHUNKS * L], f32, isOutput=False)
    cb = nc.declare_dram_parameter("cb", [128, 8], f32, isOutput=False)
    out = nc.declare_dram_parameter("out", [L, NC], f32, isOutput=True)

    HF = NC // 2  # psum-bank / fp32 matmul moving-operand free-dim limit

    # sub-chunk shapes: {7,7,2} lobes per cst tile
    SUBS = ((0, 7), (7, 7), (14, 2))

    with tile.TileContext(nc) as tc:
        with (
            tc.tile_pool(name="const", bufs=1) as cpool,
            tc.tile_pool(name="io", bufs=2) as io,
            tc.tile_pool(name="wp", bufs=2) as wpool,
            tc.tile_pool(name="wstage", bufs=2) as wstage,
            tc.tile_pool(name="abc", bufs=2) as abcp,
            tc.tile_pool(name="trig", bufs=2) as trig,
            tc.tile_pool(name="work", bufs=2) as work,
            tc.tile_pool(name="hrp", bufs=4) as hrp,
            tc.tile_pool(name="ps", bufs=5, space=bass.MemorySpace.PSUM) as ps,
            tc.tile_pool(name="zps", bufs=1, space=bass.MemorySpace.PSUM) as zps,
            tc.tile_pool(name="ops", bufs=1, space=bass.MemorySpace.PSUM) as opsp,
        ):
            cb_t = cpool.tile([128, 8], f32)
            nc.sync.dma_start(cb_t[:], cb[:])
            nrmT_t = cpool.tile([3, NC], f32)
            nc.sync.dma_start(nrmT_t[:], nrmT[:])
            wabc_t = cpool.tile([3, 3 * L], f32)
            nc.sync.dma_start(wabc_t[:], wabc[:])
            # fp32r operands must be produced by a rounding compute op (BIR
            # verifier rule) — stage f32 loads through wstage, round via copy.
            # Emission order is chunk-0 criticality: pc chain and chunk-0
            # weights first, bulk wsig/wsum last.
            pc_r = cpool.tile([4, NC], f32r)
            wsig_r = cpool.tile([128, 2048], f32r)
            wsum_r = cpool.tile([128, CHUNKS * L], f32r)
            st1 = wstage.tile([128, LPC * 128], f32, tag="wst")
            st2 = wstage.tile([128, LPC * 128], f32, tag="wst")
            nc.gpsimd.dma_start(st2[0:4, 0:NC], pc[:])
            nc.vector.tensor_copy(pc_r[:], st2[0:4, 0:NC])

            r_chunks = []
            for C in range(4):
                r_c = cpool.tile([128, NC], f32, tag=f"rfull{C}")
                r_chunks.append(r_c)
            nc.gpsimd.dma_start(r_chunks[0][:], rt[0:128, :])

            # chunk 0's weights through a dedicated staging tile, off the
            # wstage rotation so nothing upstream gates it
            wcst0_t = cpool.tile([128, LPC * 128], f32)
            nc.sync.dma_start(wcst0_t[:], wcst[:, 0:LPC * 128])
            wcst_r0 = wpool.tile([128, LPC * 128], f32r, tag="wcstr")
            nc.vector.tensor_copy(wcst_r0[:], wcst0_t[:])

            # persistent ping-pong cst tiles; pc rows staged once per buffer
            # (buffer 0 now, buffer 1 deferred into chunk 0's body)
            cst_pp = []
            for bb in range(2):
                row = []
                for k, (lo, m) in enumerate(SUBS):
                    t = cpool.tile([128, NC], f32r, tag=f"cstpp{bb}{k}")
                    if bb == 0:
                        nc.gpsimd.dma_start(t[16 * m:16 * m + 4, :].bitcast(f32),
                                            pc_r[:].bitcast(f32))
                    row.append(t)
                cst_pp.append(row)

            nc.gpsimd.dma_start(r_chunks[1][:], rt[128:256, :])
            nc.gpsimd.dma_start(st1[:], wsig[:])
            nc.vector.tensor_copy(wsig_r[:], st1[:])
            nc.gpsimd.dma_start(st2[:, NC:2 * NC], wsum[:])
            nc.vector.tensor_copy(wsum_r[:], st2[:, NC:2 * NC])
            ones_t = cpool.tile([128, HF], f32)
            nc.vector.tensor_scalar(ones_t[:], st1[:, 0:HF], 0.0, 1.0,
                                    ALU.mult, ALU.add)

            # hemisphere-mask dots in [l, n] layout: full fp32 (sign-exact)
            # abc_all[:, w*NC : (w+1)*NC] = a / b / c
            abc_all = cpool.tile([128, 3 * NC], f32)
            for hf in range(2):
                fs = hf * HF
                for wi in range(3):
                    pab = ps.tile([128, HF], f32, tag="ph")
                    nc.tensor.matmul(pab[:], wabc_t[:, wi * L:(wi + 1) * L],
                                     nrmT_t[:, fs:fs + HF], start=True, stop=True)
                    nc.vector.tensor_copy(abc_all[:, wi * NC + fs:wi * NC + fs + HF],
                                          pab[:])

            out_ps = opsp.tile([128, NC], f32)
            out_sb = cpool.tile([128, NC], f32)

            # vis' = tanh(z/2 + b2/2); sigmoid = (1+vis')/2, the 1/2 lives in
            # wsum.  vm = (1+vis')*msk.
            def _flush(p):
                zt_p, msk_p, C_p, fs_p = p
                vis = work.tile([128, HF], f32, tag="vis")
                nc.scalar.activation(vis[:], zt_p[:], AF.Tanh,
                                     bias=cb_t[:, 2:3], scale=0.5)
                vp = work.tile([128, HF], f32, tag="q1")
                nc.gpsimd.tensor_add(vp[:], vis[:], ones_t[:])
                vm = work.tile([128, HF], f32r, tag="vm")
                nc.gpsimd.tensor_mul(vm[:], vp[:], msk_p[:])
                nc.tensor.matmul(out_ps[:, fs_p:fs_p + HF],
                                 wsum_r[:, C_p * L:(C_p + 1) * L], vm[:],
                                 start=(C_p == 0), stop=(C_p == CHUNKS - 1))
                if C_p == CHUNKS - 1:
                    nc.vector.tensor_copy(out_sb[:, fs_p:fs_p + HF],
                                          out_ps[:, fs_p:fs_p + HF])
                    nc.sync.dma_start(out[:, fs_p:fs_p + HF],
                                      out_sb[:, fs_p:fs_p + HF])

            for C in range(CHUNKS):
                th = r_chunks[C % 4]
                if C == 0:
                    wcst_r = wcst_r0
                else:
                    # split halves so the urgent r_b transfers can slip
                    # between them on the shared DMA engines
                    wcst_t = wstage.tile([128, LPC * 128], f32, tag="wst")
                    cw = C * LPC * 128
                    nc.sync.dma_start(wcst_t[:, 0:LPC * 64],
                                      wcst[:, cw:cw + LPC * 64])
                    nc.sync.dma_start(wcst_t[:, LPC * 64:LPC * 128],
                                      wcst[:, cw + LPC * 64:cw + LPC * 128])
                    wcst_r = wpool.tile([128, LPC * 128], f32r, tag="wcstr")
                    nc.gpsimd.tensor_copy(wcst_r[:], wcst_t[:])

                # block-layout theta: partition = (lobe8, ct|st, s8); ONE
                # merged DMA duplicates the rows, ONE Sin per sub-chunk emits
                # the fp32r moving operand.  Issued before the abc dup: these
                # gate the matmul stream, the abc dup only gates the
                # end-of-half mask multiply.
                cst_rs = cst_pp[C % 2]
                for k, (lo, m) in enumerate(SUBS):
                    r_b = io.tile([128, NC], f32, tag="rb")
                    src = th[8 * lo:8 * (lo + m), :].unsqueeze(1)
                    src = src.broadcast_to((8 * m, 2, NC))
                    nc.sync.dma_start(r_b[0:16 * m, :], src)
                    nc.scalar.activation(cst_rs[k][0:16 * m, :],
                                         r_b[0:16 * m, :], AF.Sin,
                                         bias=cb_t[0:16 * m, 5 + (k == 2):6 + (k == 2)],
                                         scale=PI4)

                # theta prefetch lands in the chunk's DMA-quiet middle
                if C + 2 < CHUNKS:
                    nc.gpsimd.dma_start(r_chunks[(C + 2) % 4][:],
                                        rt[(C + 2) * 128:(C + 3) * 128, :])

                if C == 0:
                    # deferred non-critical staging, off the head's DMA window
                    for k, (lo, m) in enumerate(SUBS):
                        nc.gpsimd.dma_start(
                            cst_pp[1][k][16 * m:16 * m + 4, :].bitcast(f32),
                            pc_r[:].bitcast(f32))

                # s-duplication DMAs for a/b/c, split three ways so the
                # mask chain can start as soon as 'a' lands
                abc_C = abcp.tile([128, 3 * NC], f32, tag="abcC")
                for wi in range(3):
                    dup = abc_all[C * LPC:(C + 1) * LPC,
                                  wi * NC:(wi + 1) * NC].unsqueeze(1)
                    dup = dup.broadcast_to((LPC, 8, NC))
                    nc.sync.dma_start(abc_C[:, wi * NC:(wi + 1) * NC], dup)
                a_C = abc_C[:, 0 * NC:1 * NC]
                b_C = abc_C[:, 1 * NC:2 * NC]
                c_C = abc_C[:, 2 * NC:3 * NC]

                # mask-path trig in (l,s) layout, full fp32
                ct_m = trig.tile([128, NC], f32, tag="ct")
                st_m = trig.tile([128, NC], f32, tag="st")
                nc.scalar.activation(ct_m[:], th[:], AF.Sin,
                                     bias=cb_t[:, 0:1], scale=PI4)
                nc.scalar.activation(st_m[:], th[:], AF.Sin,
                                     bias=cb_t[:, 1:2], scale=PI4)

                for hf in range(2):
                    fs = hf * HF
                    # mask chain: products/sums on Pool (TensorTensor only —
                    # TSP/is_gt are not legal Pool opcodes), compare on DVE
                    q1 = work.tile([128, HF], f32, tag="q1")
                    q2 = work.tile([128, HF], f32, tag="q2")
                    q3 = work.tile([128, HF], f32, tag="q3")
                    msk = work.tile([128, HF], f32, tag="msk")
                    nc.gpsimd.tensor_mul(q1[:], ct_m[:, fs:fs + HF],
                                         a_C[:, fs:fs + HF])
                    nc.gpsimd.tensor_mul(q2[:], st_m[:, fs:fs + HF],
                                         b_C[:, fs:fs + HF])
                    nc.gpsimd.tensor_add(q3[:], q1[:], q2[:])
                    nc.gpsimd.tensor_add(q2[:], q3[:], c_C[:, fs:fs + HF])
                    nc.vector.tensor_scalar(msk[:], q2[:], TINY, 0.0,
                                            ALU.is_gt, ALU.bypass)
                    zt = zps.tile([128, HF], f32, tag="zt")
                    for j16 in range(LPC):
                        k = min(j16 // 7, 2)
                        kv = 16 * (7 if k < 2 else 2) + 4
                        ph = ps.tile([128, HF], f32, tag="ph")
                        nc.tensor.matmul(ph[:],
                                         wcst_r[0:kv, j16 * 128:(j16 + 1) * 128],
                                         cst_rs[k][0:kv, fs:fs + HF],
                                         start=True, stop=True)
                        hr = hrp.tile([128, HF], f32r, tag="hr")
                        if j16 % 8 in (0, 2, 4, 5, 7) and not (hf == 1 and j16 == 12):
                            nc.vector.tensor_scalar(hr[:], ph[:], 0.0, 0.0,
                                                    ALU.max, ALU.bypass)
                        else:
                            nc.scalar.activation(hr[:], ph[:], AF.Relu,
                                                 bias=cb_t[:, 3:4])
                        nc.tensor.matmul(zt[:, :],
                                         wsig_r[:, j16 * 128:(j16 + 1) * 128],
                                         hr[:],
                                         start=(j16 == 0), stop=(j16 == LPC - 1))
                    _flush((zt, msk, C, fs))

    nc.compile()

    # All activation funcs used (Sin, Relu, Tanh) live together in act table
    # set 18 (silu_and_others), but the auto-inserted loads pick per-func
    # canonical sets (Sin->9, Tanh->0) and flip-flop every chunk at 1.28us a
    # load.  Keep one load of set 18 up front, drop the rest.
    SET_ALL = 18
    first = True
    for b in nc.m.functions[0].blocks:
        kept = []
        for i in b.instructions:
            if isinstance(i, mybir.InstLoadActFuncSet):
                if first:
                    i.act_func_set_id = SET_ALL
                    first = False
                    kept.append(i)
            else:
                kept.append(i)
        b.instructions[:] = kept
    return nc


def _host_constants(points, normals, root_rot, lgtSGLobes, lgtSGLambdas,
                    W1, b1, W2, b2):
    f8 = np.float64
    lob = lgtSGLobes.astype(f8)
    l = lob / (np.linalg.norm(lob, axis=-1, keepdims=True) + TINY)
    z = np.zeros_like(l)
    z[:, 2] = 1.0
    U = np.cross(z, l)
    U = U / (np.linalg.norm(U, axis=-1, keepdims=True) + TINY)
    V = np.cross(l, U)
    V = V / (np.linalg.norm(V, axis=-1, keepdims=True) + TINY)
    sharp = lgtSGLambdas[:, 0].astype(f8)
    r_phi = np.minimum(np.arccos(1.0 - 1.0 / sharp), np.pi / 3.0)
    sp, cp = np.sin(r_phi), np.cos(r_phi)

    Wd = root_rot.astype(f8) @ W1[3:].astype(f8)          # [3,H]
    A = sp[:, None] * (U @ Wd)                             # [L,H]
    B = sp[:, None] * (V @ Wd)
    C = cp[:, None] * (l @ Wd)
    W1p = W1[:3].astype(f8)                                # [3,H]
    b1f = b1.astype(f8)
    w2 = W2[:, 0].astype(f8)
    w_l = np.exp(sharp * (cp - 1.0))
    scale_l = 0.5 * w_l / (S * w_l + TINY)   # 0.5: sigmoid = (1+tanh)/2
    spU = sp[:, None] * U
    spV = sp[:, None] * V
    cpl = cp[:, None] * l

    # wcst: [128, L*128]; col = l*128 + s*16 + h.  Sub-chunk layout {7,7,2}
    # within each 16-lobe super-chunk; per-lobe rows in its cst tile
    # (ct/st interleaved: the theta-duplication DMA repeats each source row
    # twice consecutively): ct: 2*(8*jj+s) -> -A, st: 2*(8*jj+s)+1 -> -B,
    # pc: 16*m..16*m+4 -> W1p,b1-C.
    wcstZ = np.zeros((128, 128, 128), f8)
    wcstV = wcstZ.reshape(128, L, 8, H)
    for ll in range(L):
        pos = ll % LPC
        k = min(pos // 7, 2)
        jj = pos - 7 * k
        m = 7 if k < 2 else 2
        for s in range(8):
            wcstV[2 * (8 * jj + s), ll, s, :] = -A[ll]
            wcstV[2 * (8 * jj + s) + 1, ll, s, :] = -B[ll]
        for d in range(3):
            wcstV[16 * m + d, ll, :, :] = W1p[d]
        wcstV[16 * m + 3, ll, :, :] = (b1f - C[ll])[None, :]

    # wabc: [3, 3*L]; per-lobe columns (no s duplication)
    wabc = np.concatenate([spU.T, spV.T, cpl.T], axis=1)

    # wsig: [128, 16*128]; per-lobe block j: cols j*128 + (8*j' + s') =
    # w2[h]*delta(s,s')*delta(j,j') — all 16 lobes of a chunk accumulate into
    # one [128, HF] z tile (row 8*j + s)
    wsig = np.zeros((8, H, 16, 16, 8), f8)
    for p in range(16):
        for s in range(8):
            wsig[s, :, p, p, s] = w2
    # wsum: per-chunk [128, L] blocks; block cc maps chunk-local lobe lp to
    # global output column cc*16+lp (zero elsewhere).
    wsum = np.zeros((LPC, 8, CHUNKS, L), f8)
    for cc in range(CHUNKS):
        for lp in range(LPC):
            wsum[lp, :, cc, cc * LPC + lp] = scale_l[cc * LPC + lp]

    cbias = np.zeros((128, 8), f8)
    s_of_p = np.arange(128) % 8
    # ACT Sin LUT domain is [-pi, pi]; input is r*pi/4 + bias with r in [0,1),
    # so shift each s-row by a full period where needed to stay in range.
    cos_bias = s_of_p * (np.pi / 4.0) + np.pi / 2.0 - 2.0 * np.pi * (s_of_p >= 2)
    sin_bias = s_of_p * (np.pi / 4.0) - 2.0 * np.pi * (s_of_p >= 4)
    cbias[:, 0] = cos_bias
    cbias[:, 1] = sin_bias
    cbias[:, 2] = float(b2[0]) / 2.0                      # tanh bias = b2/2
    cbias[:, 3] = 0.0                                     # relu bias
    # sub-chunk tile layouts (ct/st interleaved): row q=2u+t holds theta of
    # u (s = u%8); t=0 -> cos phase, t=1 -> sin phase.  col5 for m=7 (q<112),
    # col6 for m=2 (q<32).
    p = np.arange(128)
    s_il = (p // 2) % 8
    cos_il = s_il * (np.pi / 4.0) + np.pi / 2.0 - 2.0 * np.pi * (s_il >= 2)
    sin_il = s_il * (np.pi / 4.0) - 2.0 * np.pi * (s_il >= 4)
    il = np.where(p % 2 == 0, cos_il, sin_il)
    cbias[:, 5] = np.where(p < 112, il, 0.0)
    cbias[:, 6] = np.where(p < 32, il, 0.0)

    f32 = np.float32
    return dict(
        wcst=np.ascontiguousarray(wcstZ.reshape(128, L * 128), f32),
        wabc=np.ascontiguousarray(wabc, f32),
        wsig=np.ascontiguousarray(wsig.reshape(128, 2048), f32),
        wsum=np.ascontiguousarray(wsum.reshape(128, CHUNKS * L), f32),
        cb=np.ascontiguousarray(cbias, f32),
    )


def _make_in_maps(inputs):
    const = _host_constants(inputs["points"], inputs["normals"],
                            inputs["root_rot"], inputs["lgtSGLobes"],
                            inputs["lgtSGLambdas"], inputs["W1"],
                            inputs["b1"], inputs["W2"], inputs["b2"])
    f32 = np.float32
    r_t = np.asarray(inputs["r_theta_random"], f32).transpose(1, 2, 0).reshape(L * S, N)
    pT = np.asarray(inputs["points"], f32).T
    nT = np.asarray(inputs["normals"], f32).T
    ones = np.ones((1, N), f32)
    in_maps = []
    for c in range(NCORES):
        sl = slice(c * NC, (c + 1) * NC)
        m = dict(const)
        m["rt"] = np.ascontiguousarray(r_t[:, sl])
        m["nrmT"] = np.ascontiguousarray(nT[:, sl])
        m["pc"] = np.ascontiguousarray(
            np.concatenate([pT[:, sl], ones[:, sl]], axis=0))
        in_maps.append(m)
    return in_maps


def kernel(points, normals, root_rot, lgtSGLobes, lgtSGLambdas,
           r_theta_random, W1, b1, W2, b2):
    global _PROG
    from concourse.bass_utils import run_bass_kernel_spmd

    if _PROG is None:
        _PROG = _build_program()
    nc = _PROG

    in_maps = _make_in_maps(dict(
        points=points, normals=normals, root_rot=root_rot,
        lgtSGLobes=lgtSGLobes, lgtSGLambdas=lgtSGLambdas,
        r_theta_random=r_theta_random, W1=W1, b1=b1, W2=W2, b2=b2))

    res = run_bass_kernel_spmd(nc, in_maps, list(range(NCORES)))

    f32 = np.float32
    out_full = np.empty((N, L), f32)
    for c in range(NCORES):
        out_full[c * NC:(c + 1) * NC, :] = res.results[c]["out"].T
    return out_full
